# revision 1
# baseline (speedup 1.0000x reference)
"""Causal multi-head attention block (QKV proj + causal softmax attention + out proj)
for Trainium2, sharded over 8 NeuronCores.

Sharding: tensor-parallel over heads x data-parallel over batch.
  core (b, g) for b in {0,1}, g in {0..3}: batch b, head group g (4 heads of 16).
  Each core computes its 4 heads' attention output slice and a partial
  output projection (row-parallel W_O); host sums the 4 partials per batch.

Device layout: everything is computed in "transposed" orientation so no
on-chip transposes are needed anywhere:
  - host passes x^T, W_qkv^T (sliced), W_O^T (sliced) per core
  - Q^T,K^T = (W^T)^T @ x^T via PE;  V in natural [t,d] layout (+ ones column)
  - S^T[k,q] = (K^T)^T @ Q^T; exp on ScalarE (no max-subtraction needed:
    scores are ~N(0,1), exp is safe in fp32); causal masking by zeroing the
    diagonal P^T tile with a gpsimd affine_select after exp, and by only
    computing/accumulating the causally-valid column range per k-tile
  - O^T[d,q] (+ sum row from the ones column) = V_aug^T @ P^T, accumulated
    only over the causally-valid column range per k-tile
  - normalize with reciprocal_approx_fast + gpsimd partition_broadcast
  - partial_out[t,o] = (attn^T)^T @ W_O^T

All matmul operands use dtype float32r (fp32 bits, PE 'replicated' datapath):
full bf16-rate matmul at ~1e-4 relative accuracy.
"""

import sys

sys.path.insert(0, "/opt/trn_rl_repo")

import numpy as np

import concourse.bacc as bacc
import concourse.mybir as mybir
import concourse.tile as tile
from concourse import bass_utils

B, T, C = 2, 2048, 1024
H, DK = 16, 64
G = 4  # tensor-parallel head groups
HG = H // G  # heads per core
WQK = 2 * HG * DK  # 512: Q+K cols per core in wqkvT
WV = HG * DK  # 256: V cols per core
N_CORES = 8
F32 = mybir.dt.float32
F32R = mybir.dt.float32r

TCH = 4  # t chunks of 512 for N-dim of matmuls
CK = C // 128  # 8 contraction chunks
NT = T // 128  # 16 t-tiles
QCH = 512  # q chunk


def _emit(nc, xT, wqkvT, woT, ones, out, taps=None):
    with tile.TileContext(nc) as tc:
        with (
            tc.tile_pool(name="persist", bufs=1) as persist,
            tc.tile_pool(name="pt", bufs=5) as pt_pool,
            tc.tile_pool(name="small", bufs=4) as small_pool,
            tc.tile_pool(name="ob", bufs=5) as ob_pool,
            tc.tile_pool(name="qkv_ps", bufs=2, space="PSUM") as qkv_ps,
            tc.tile_pool(name="st_ps", bufs=3, space="PSUM") as st_ps,
            tc.tile_pool(name="ot_ps", bufs=2, space="PSUM") as ot_ps,
            tc.tile_pool(name="wo_ps", bufs=1, space="PSUM") as wo_ps,
        ):
            xT_all = persist.tile([128, CK, T], F32R, tag="xT_all")
            w_all = persist.tile([128, CK, 3 * WV], F32R, tag="w_all")
            woT_all = persist.tile([128, 2, C], F32R, tag="woT_all")
            qkT = persist.tile([128, 4, T], F32R, tag="qkT")
            vaug = persist.tile([128, NT, HG, DK + 1], F32R, tag="vaug")
            attnT = persist.tile([128, 2, T], F32R, tag="attnT")

            # weights + first x t-chunk interleaved, then remaining x t-chunks;
            # w and x go through different engines' DMA queues so the first
            # c-chunks arrive in parallel
            for k in range(CK):
                nc.sync.dma_start(
                    w_all[:, k, 0:WQK], wqkvT[k * 128 : (k + 1) * 128, 0:WQK]
                )
                nc.scalar.dma_start(
                    xT_all[:, k, 0:QCH], xT[k * 128 : (k + 1) * 128, 0:QCH]
                )
            # ones column (last col) for the softmax-denominator rows
            nc.sync.dma_start(vaug[:, :, :, DK : DK + 1], ones[:])
            # V weight columns arrive after the Q/K columns
            for k in range(CK):
                nc.sync.dma_start(
                    w_all[:, k, WQK:], wqkvT[k * 128 : (k + 1) * 128, WQK:]
                )
            for tch in range(1, TCH):
                for k in range(CK):
                    nc.sync.dma_start(
                        xT_all[:, k, tch * QCH : (tch + 1) * QCH],
                        xT[k * 128 : (k + 1) * 128, tch * QCH : (tch + 1) * QCH],
                    )
            for j in range(2):
                nc.sync.dma_start(woT_all[:, j, :], woT[j * 128 : (j + 1) * 128, :])

            def qk_mm(ps, j, tch, k):
                nc.tensor.matmul(
                    ps[:],
                    w_all[:, k, j * 128 : (j + 1) * 128],
                    xT_all[:, k, tch * QCH : (tch + 1) * QCH],
                    start=(k == 0),
                    stop=(k == CK - 1),
                )

            def qk_copy(ps, j, tch):
                nc.vector.tensor_copy(qkT[:, j, tch * QCH : (tch + 1) * QCH], ps[:])

            def v_mm(ps, ti, k):
                nc.tensor.matmul(
                    ps[:],
                    xT_all[:, k, ti * 128 : (ti + 1) * 128],
                    w_all[:, k, WQK : WQK + WV],
                    start=(k == 0),
                    stop=(k == CK - 1),
                )

            def v_copy(ps, ti):
                nc.vector.tensor_copy(
                    vaug[:, ti, :, 0:DK],
                    ps[:].rearrange("p (h d) -> p h d", h=HG),
                )

            def emit_qk(j, tch):
                # qkT[:, j, t-chunk] = W[:, j*128:(j+1)*128].T @ xT
                ps = qkv_ps.tile([128, QCH], F32, tag="mm")
                for k in range(CK):
                    qk_mm(ps, j, tch, k)
                qk_copy(ps, j, tch)

            def emit_v(ti):
                # vaug[:, ti, h, 1:] = xT[:, ti-tile].T @ Wv  -> [128 t, 256]
                ps = qkv_ps.tile([128, WV], F32, tag="mm")
                for k in range(CK):
                    v_mm(ps, ti, k)
                v_copy(ps, ti)

            def emit_qkv_chunk0():
                # DMA-bound startup: k-outer over all four W tiles and the
                # first two V tiles so the PE consumes each arriving c-chunk
                # of x/w with ~6 matmuls instead of stalling per psum group.
                ps_j = {
                    j: qkv_ps.tile([128, QCH], F32, tag="mm", name=f"ps_j{j}")
                    for j in (0, 2)
                }
                ps_j[1] = ot_ps.tile([128, QCH], F32, tag="ot", name="ps_j1")
                ps_j[3] = ot_ps.tile([128, QCH], F32, tag="ot", name="ps_j3")
                for k in range(CK):
                    for j in (0, 2, 1, 3):
                        qk_mm(ps_j[j], j, 0, k)
                for j in (0, 2, 1, 3):
                    qk_copy(ps_j[j], j, 0)
                for ti in (0, 1, 2, 3):
                    emit_v(ti)

            def emit_head_chunk(h, q0, qlen):
                prow = (h % 2) * 64
                QT_h = qkT[prow : prow + 64, h // 2, :]
                KT_h = qkT[prow : prow + 64, 2 + h // 2, :]
                if True:
                    nk = (q0 + qlen) // 128
                    ot = ot_ps.tile([DK + 1, QCH], F32, tag="ot", name="ot")[:, 0:qlen]
                    for k in range(nk):
                        k0 = k * 128
                        delta = k0 - q0
                        d0 = max(delta, 0)
                        # fp32r matmuls below N=256 fall back to 4 cycles/row;
                        # widen the window so ST/PV keep N>=256 (the extra
                        # invalid columns are zeroed by the affine_select)
                        w0 = min(d0, qlen - 256) if qlen >= 256 else d0
                        aw = d0 + 128 - w0  # affine width: ragged + extension
                        st = st_ps.tile([128, QCH], F32, tag="st", name="st")[:, 0:qlen]
                        nc.tensor.matmul(
                            st[:, w0:qlen],
                            KT_h[:, k0 : k0 + 128],
                            QT_h[:, q0 + w0 : q0 + qlen],
                            start=True,
                            stop=True,
                        )
                        pt = pt_pool.tile([128, QCH], F32R, tag="pt", name="pt")[:, 0:qlen]
                        nc.scalar.activation(
                            pt[:, w0:qlen], st[:, w0:qlen],
                            mybir.ActivationFunctionType.Exp,
                            scale=float(1.0 / np.sqrt(DK)),
                        )
                        if delta >= 0:
                            # zero entries with q_global < k_global (covers the
                            # widened zone [w0:d0) entirely plus the triangle)
                            nc.gpsimd.affine_select(
                                out=pt[:, w0 : w0 + aw],
                                in_=pt[:, w0 : w0 + aw],
                                compare_op=mybir.AluOpType.is_ge,
                                fill=0.0,
                                base=w0 - delta,
                                pattern=[[1, aw]],
                                channel_multiplier=-1,
                            )
                        # accumulate only the causally-valid columns; columns
                        # < w0 get no contribution from this k-tile (they are
                        # exactly zero), and k=0 (w0=0) initializes the bank.
                        nc.tensor.matmul(
                            ot[:, w0:qlen],
                            vaug[:, k, h, :],
                            pt[:, w0:qlen],
                            start=(k == 0),
                            stop=(k == nk - 1),
                        )
                    # custom-DVE recip needs a base-partition-0 source; stage
                    # the sums row (psum partition 64) through sbuf partition 0
                    sums_sb = small_pool.tile([1, QCH], F32, tag="sums", name="sums_sb")[:, 0:qlen]
                    nc.vector.tensor_copy(sums_sb[:], ot[DK : DK + 1, :])
                    recip = small_pool.tile([1, QCH], F32, tag="recip", name="recip")[:, 0:qlen]
                    nc.vector.reciprocal_approx_fast(out=recip[:], in_=sums_sb[:])
                    rb = small_pool.tile([64, QCH], F32, tag="rb", name="rb")[:, 0:qlen]
                    nc.gpsimd.partition_broadcast(rb[:], recip[:])
                    nc.vector.tensor_tensor(
                        attnT[prow : prow + 64, h // 2, q0 : q0 + qlen],
                        ot[0:DK, :],
                        rb[:],
                        mybir.AluOpType.mult,
                    )

            def emit_wo(ti, alternate=False, pool=None, ptag="wo"):
                for oc in range(2):
                    ps = (pool or wo_ps).tile([128, QCH], F32, tag=ptag, name="wops")
                    for j in range(2):
                        nc.tensor.matmul(
                            ps[:],
                            attnT[:, j, ti * 128 : (ti + 1) * 128],
                            woT_all[:, j, oc * QCH : (oc + 1) * QCH],
                            start=(j == 0),
                            stop=(j == 1),
                        )
                    ob = ob_pool.tile([128, QCH], F32, tag="ob")
                    if alternate and (ti + oc) % 2 == 0:
                        nc.scalar.copy(ob[:], ps[:])
                    else:
                        nc.vector.tensor_copy(ob[:], ps[:])
                    nc.sync.dma_start(
                        out[ti * 128 : (ti + 1) * 128, oc * QCH : (oc + 1) * QCH],
                        ob[:],
                    )

            # Pipelined emission over t-chunks. Attention segments are paced
            # by ScalarE's exp, so each segment's head chunks are interleaved
            # with dense PE filler work: the NEXT chunk's QKV/V projections
            # and the PREVIOUS chunk's W_O tiles.
            emit_qkv_chunk0()
            for seg in range(TCH):
                fillers = []
                nxt = seg + 1
                if nxt < TCH:
                    for j in (0, 2, 1, 3):
                        fillers.append(lambda j=j: emit_qk(j, nxt))
                    for ti in range(4 * nxt, 4 * nxt + 4):
                        fillers.append(lambda ti=ti: emit_v(ti))
                if seg >= 1:
                    for ti in range(4 * (seg - 1), 4 * (seg - 1) + 4):
                        fillers.append(lambda ti=ti: emit_wo(ti))
                per = (len(fillers) + HG - 1) // HG if fillers else 0
                for h in range(HG):
                    emit_head_chunk(h, seg * QCH, QCH)
                    for f in fillers[h * per : (h + 1) * per]:
                        f()
            # tail: the last chunk's W_O through the now-idle 2-slot qkv pool,
            # copies alternating between ScalarE and VectorE
            for ti in range(4 * (TCH - 1), 4 * (TCH - 1) + 4):
                emit_wo(ti, alternate=True, pool=qkv_ps, ptag="mm")

            if taps is not None:
                nc.sync.dma_start(taps["qkT"][:], qkT[:])
                nc.sync.dma_start(taps["vaug"][:], vaug[:])
                nc.sync.dma_start(taps["attnT"][:], attnT[:])


_CACHE = {}


def _build():
    if "nc" in _CACHE:
        return _CACHE["nc"]
    nc = bacc.Bacc("TRN2", debug=False, num_devices=N_CORES)
    xT = nc.dram_tensor("xT", [C, T], F32R, kind="ExternalInput").ap()
    wqkvT = nc.dram_tensor("wqkvT", [C, 3 * WV], F32R, kind="ExternalInput").ap()
    woT = nc.dram_tensor("woT", [2 * 128, C], F32R, kind="ExternalInput").ap()
    ones = nc.dram_tensor("ones", [128, NT, HG, 1], F32R, kind="ExternalInput").ap()
    out = nc.dram_tensor("out", [T, C], F32, kind="ExternalOutput").ap()
    _emit(nc, xT, wqkvT, woT, ones, out)
    nc.compile()
    _CACHE["nc"] = nc
    return nc


_ONES = np.ones((128, NT, HG, 1), dtype=np.float32)


def _shard_inputs(x, W_QKV, W_O):
    """Build the 8 per-core input maps. core = b*G + g."""
    in_maps = []
    W_Q, W_K, W_V = W_QKV[0:C], W_QKV[C : 2 * C], W_QKV[2 * C : 3 * C]
    for b in range(B):
        xT_b = np.ascontiguousarray(x[b].T)  # [C, T]
        for g in range(G):
            sl = slice(g * HG * DK, (g + 1) * HG * DK)
            w_g = np.concatenate([W_Q[sl], W_K[sl], W_V[sl]], axis=0)  # [768, C]
            wqkvT_g = np.ascontiguousarray(w_g.T)  # [C, 768]
            woT_g = np.ascontiguousarray(W_O[:, sl].T)  # [256, C]
            in_maps.append(
                {"xT": xT_b, "wqkvT": wqkvT_g, "woT": woT_g, "ones": _ONES}
            )
    return in_maps


def kernel(x, W_QKV, W_O):
    x = np.asarray(x, dtype=np.float32)
    W_QKV = np.asarray(W_QKV, dtype=np.float32)
    W_O = np.asarray(W_O, dtype=np.float32)
    nc = _build()
    in_maps = _shard_inputs(x, W_QKV, W_O)
    res = bass_utils.run_bass_kernel_spmd(
        nc, in_maps, core_ids=list(range(N_CORES))
    )
    out = np.zeros((B, T, C), dtype=np.float32)
    for b in range(B):
        for g in range(G):
            out[b] += res.results[b * G + g]["out"]
    return out



# revision 36
# speedup vs baseline: 1.2826x; 1.2826x over previous
"""Causal multi-head attention block (QKV proj + causal softmax attention + out proj)
for Trainium2, sharded over 8 NeuronCores.

Sharding: tensor-parallel over heads x data-parallel over batch.
  core (b, g) for b in {0,1}, g in {0..3}: batch b, head group g (4 heads of 16).
  Each core computes its 4 heads' attention output slice and a partial
  output projection (row-parallel W_O); host sums the 4 partials per batch.

All on-chip attention math runs in bf16 operands with fp32 PSUM accumulation
(1 cycle/row on the PE at any moving size, so no min-width padding is needed
anywhere):
  - Q^T,K^T = (W^T)^T @ x^T on the PE, stored bf16; V stored per k-tile as
    [t,d] bf16 with an appended ones column (softmax denominator).
  - S^T[k,q] = (K^T)^T @ Q^T per 128-wide k-tile over the causally valid
    q-span; two k-tiles share one 2-bank PSUM slab so ScalarE exps them in a
    single instruction. Causal raggedness of the diagonal tiles is zeroed
    post-exp with a gpsimd affine_select on the bf16 P tile.
  - O[q,d] (+ denominator column) = P^T.T @ V_aug: the P tile is the
    *stationary* operand, so the output has a full 128 q-partitions and only
    65 moving columns -- half the PE cycles of the [d,q] formulation.
  - normalize: per-partition reciprocal of the denominator column +
    tensor_scalar multiply (no partition broadcast needed).
  - attn [q,d] -> attnT [d,q] via PE transposes (128x128, bf16), then
    partial_out[t,o] = attnT.T @ W_O^T.

Scheduling: ScalarE's exp paces the attention inner loop, so the PE stream is
software-pipelined -- each slab's PV matmuls run one slab behind its ST/exp,
carried across head and seg boundaries -- and the gaps are filled with the
next seg's QKV projection and earlier segs' W_O tiles. Input DMAs are batched
(multi-tile access patterns) to keep the single HWDGE generator off the
critical path.
"""

import sys

sys.path.insert(0, "/opt/trn_rl_repo")

import numpy as np
import ml_dtypes

import concourse.bacc as bacc
import concourse.mybir as mybir
import concourse.tile as tile
from concourse import bass_utils

B, T, C = 2, 2048, 1024
H, DK = 16, 64
G = 4  # tensor-parallel head groups
HG = H // G  # heads per core
WQ = HG * DK  # 256 Q (=K=V) cols per core
N_CORES = 8
F32 = mybir.dt.float32
F16 = mybir.dt.float16
BF = mybir.dt.bfloat16

TCH = 4  # t chunks of 512 (attention segs)
CK = C // 128  # 8 contraction chunks
NT = T // 128  # 16 k/t-tiles
QCH = 512  # q chunk


def _emit(nc, xT, wqkvT, woT, ident, out):
    with tile.TileContext(nc) as tc:
        with (
            tc.tile_pool(name="persist", bufs=1) as persist,
            tc.tile_pool(name="pt", bufs=12) as pt_pool,
            tc.tile_pool(name="small", bufs=6) as small_pool,
            tc.tile_pool(name="ob", bufs=4) as ob_pool,
            tc.tile_pool(name="st_ps", bufs=2, space="PSUM") as st_ps,
            tc.tile_pool(name="o_ps", bufs=2, space="PSUM") as o_ps,
            tc.tile_pool(name="qkv_ps", bufs=2, space="PSUM") as qkv_ps,
        ):
            xT_all = persist.tile([128, CK, T], BF, tag="xT_all")
            w_all = persist.tile([128, CK, 3 * WQ], BF, tag="w_all")
            woT_all = persist.tile([128, 2, C], BF, tag="woT_all")
            qkT = persist.tile([128, 4, T], BF, tag="qkT")
            vaug = persist.tile([128, NT, HG, DK + 1], BF, tag="vaug")
            attnT = persist.tile([128, 2, T], BF, tag="attnT")
            stage = persist.tile([128, 2, 4, 128], BF, tag="stage")
            ident_sb = persist.tile([128, 128], BF, tag="ident_sb")

            # Batched input DMAs: the first c-chunks of W_qk / x land in small
            # pieces (so the PE can start ~1.2us in and stay fed through the
            # p-state ramp); later chunks ride one multi-tile DMA each.
            for k0, k1 in ((0, 1), (1, 3), (3, 5), (5, CK)):
                w_src = wqkvT[k0 * 128 : k1 * 128, 0 : 2 * WQ]
                x_src = xT[k0 * 128 : k1 * 128, 0:QCH]
                if k1 - k0 > 1:
                    w_src = w_src.rearrange("(k p) c -> p k c", p=128)
                    x_src = x_src.rearrange("(k p) c -> p k c", p=128)
                nc.sync.dma_start(w_all[:, k0:k1, 0 : 2 * WQ], w_src)
                nc.scalar.dma_start(xT_all[:, k0:k1, 0:QCH], x_src)
            nc.sync.dma_start(
                w_all[:, :, 2 * WQ :],
                wqkvT[:, 2 * WQ :].rearrange("(k p) c -> p k c", p=128),
            )
            nc.sync.dma_start(ident_sb[:], ident[:])
            # remaining x arrives t-chunk by t-chunk, in the order the QKV
            # fillers consume it
            for tch in range(1, TCH):
                nc.scalar.dma_start(
                    xT_all[:, :, tch * QCH : (tch + 1) * QCH],
                    xT[:, tch * QCH : (tch + 1) * QCH].rearrange(
                        "(k p) c -> p k c", p=128
                    ),
                )
            nc.sync.dma_start(
                woT_all[:], woT[:].rearrange("(j p) c -> p j c", p=128)
            )
            nc.gpsimd.memset(vaug[:, :, :, DK : DK + 1], 1.0)

            def qk_mm(ps, j, tch, k):
                nc.tensor.matmul(
                    ps[:],
                    w_all[:, k, j * 128 : (j + 1) * 128],
                    xT_all[:, k, tch * QCH : (tch + 1) * QCH],
                    start=(k == 0),
                    stop=(k == CK - 1),
                )

            def emit_qk(j, tch):
                ps = qkv_ps.tile([128, QCH], F32, tag="mm", name="ps_qk")
                for k in range(CK):
                    qk_mm(ps, j, tch, k)
                nc.vector.tensor_copy(qkT[:, j, tch * QCH : (tch + 1) * QCH], ps[:])

            def emit_v(ti):
                ps = qkv_ps.tile([128, WQ], F32, tag="mm", name="ps_v")
                for k in range(CK):
                    nc.tensor.matmul(
                        ps[:],
                        xT_all[:, k, ti * 128 : (ti + 1) * 128],
                        w_all[:, k, 2 * WQ : 3 * WQ],
                        start=(k == 0),
                        stop=(k == CK - 1),
                    )
                nc.vector.tensor_copy(
                    vaug[:, ti, :, 0:DK],
                    ps[:].rearrange("p (h d) -> p h d", h=HG),
                )

            def emit_qkv_chunk0():
                # k-outer over all four Q/K tiles so the PE consumes each
                # arriving c-chunk of x/w immediately; copies split DVE/Act.
                ps_j = {
                    j: qkv_ps.tile([128, QCH], F32, tag="mm", name=f"ps_j{j}")
                    for j in (0, 1)
                }
                ps_j[2] = st_ps.tile([128, QCH], F32, tag="st", name="ps_j2")
                ps_j[3] = st_ps.tile([128, QCH], F32, tag="st", name="ps_j3")
                for k in range(CK):
                    for j in (0, 2, 1, 3):
                        qk_mm(ps_j[j], j, 0, k)
                for j, eng in ((0, nc.vector.tensor_copy), (2, nc.scalar.copy),
                               (1, nc.vector.tensor_copy), (3, nc.scalar.copy)):
                    eng(qkT[:, j, 0:QCH], ps_j[j][:])

            def emit_wo(ti, scalar_copy=False, split_dma=False):
                ob = ob_pool.tile([128, 2, QCH], F16, tag="ob")
                for oc in range(2):
                    ps = qkv_ps.tile([128, QCH], F32, tag="mm", name="ps_wo")
                    for j in range(2):
                        nc.tensor.matmul(
                            ps[:],
                            attnT[:, j, ti * 128 : (ti + 1) * 128],
                            woT_all[:, j, oc * QCH : (oc + 1) * QCH],
                            start=(j == 0),
                            stop=(j == 1),
                        )
                    if scalar_copy and oc == 0:
                        nc.scalar.copy(ob[:, oc, :], ps[:])
                    else:
                        nc.vector.tensor_copy(ob[:, oc, :], ps[:])
                    if split_dma:
                        # last tile: start each half's store as soon as its
                        # copy lands
                        nc.sync.dma_start(
                            out[
                                ti * 128 : (ti + 1) * 128,
                                oc * QCH : (oc + 1) * QCH,
                            ],
                            ob[:, oc, :],
                        )
                if not split_dma:
                    nc.sync.dma_start(
                        out[ti * 128 : (ti + 1) * 128, :],
                        ob[:].rearrange("p a b -> p (a b)"),
                    )

            def emit_tr1(s, hp, qt, scalar_copy=False):
                # single 128x128 PE transpose; each gets its own PSUM slot in
                # the o ring (a second transpose into the same bank would
                # clear the first: matmul start resets the whole bank)
                tr = o_ps.tile([128, 128], BF, tag="o", name="tr1")
                nc.tensor.transpose(tr[:], stage[:, hp, qt, :], ident_sb[:])
                cp = nc.scalar.copy if scalar_copy else nc.vector.tensor_copy
                cp(
                    attnT[:, hp, s * QCH + qt * 128 : s * QCH + (qt + 1) * 128],
                    tr[:],
                )

            # filler machinery: one slab-slot at a time between attention
            # work, paced evenly across the seg's slots by a credit counter
            fillers = []
            trq = []
            pace = {"credit": 0.0, "rate": 0.0}

            def drain():
                if trq:
                    trq.pop(0)()
                pace["credit"] += pace["rate"]
                while fillers and pace["credit"] >= 1.0:
                    fillers.pop(0)()
                    pace["credit"] -= 1.0

            # Attention: each head's ST/exp/mask slabs stream first (the P
            # tiles stay in SBUF); the PV for each q-subtile then runs as one
            # contiguous PSUM accumulation chain (a bank's groups must never
            # interleave: matmul start resets the whole bank). The previous
            # head's four PV chains are interleaved between this head's slabs.
            pieces = []

            def pv_piece(s, h, qt, pts):
                def go():
                    o_t = o_ps.tile([128, DK + 1], F32, tag="o", name="o_t")
                    for kk in range(4 * s + qt + 1):
                        nc.tensor.matmul(
                            o_t[:],
                            pts[kk // 2][:, kk % 2, 128 * qt : 128 * (qt + 1)],
                            vaug[:, kk, h, :],
                            start=(kk == 0),
                            stop=(kk == 4 * s + qt),
                        )
                    rc = small_pool.tile([128, 1], F32, tag="rc", name="rc")
                    nc.vector.reciprocal(rc[:], o_t[:, DK : DK + 1])
                    nc.vector.tensor_scalar(
                        stage[:, h // 2, qt, (h % 2) * DK : (h % 2 + 1) * DK],
                        o_t[:, 0:DK],
                        rc[:],
                        None,
                        mybir.AluOpType.mult,
                    )
                    if h % 2 == 1 and qt % 2 == 1:
                        sc = s == TCH - 1 and h == HG - 1
                        emit_tr1(s, h // 2, qt - 1, scalar_copy=sc)
                        emit_tr1(s, h // 2, qt, scalar_copy=sc)
                        if sc and qt == 1:
                            # q-tiles 12/13 complete: W_O while the last
                            # chains still run
                            emit_wo(12, scalar_copy=True)
                            emit_wo(13, scalar_copy=True)
                        if sc and qt == 3:
                            emit_wo(14, scalar_copy=True)
                            emit_wo(15, scalar_copy=True, split_dma=True)
                return go

            def emit_head(s, h):
                prow = (h % 2) * 64
                QT_h = qkT[prow : prow + 64, h // 2, :]
                KT_h = qkT[prow : prow + 64, 2 + h // 2, :]
                q0 = s * QCH
                nsl = 2 * (s + 1)
                pts = []
                for sl in range(nsl):
                    st = st_ps.tile([128, 2, QCH], F32, tag="st", name="st")
                    pt = pt_pool.tile([128, 2, QCH], BF, tag="pt", name="pt")
                    d0s = []
                    for half in (0, 1):
                        kk = 2 * sl + half
                        d0 = max(0, 128 * kk - q0)
                        d0s.append(d0)
                        nc.tensor.matmul(
                            st[:, half, d0:QCH],
                            KT_h[:, 128 * kk : 128 * (kk + 1)],
                            QT_h[:, q0 + d0 : q0 + QCH],
                            start=True,
                            stop=True,
                        )
                    d0m = d0s[0]
                    nc.scalar.activation(
                        pt[:, :, d0m:QCH],
                        st[:, :, d0m:QCH],
                        mybir.ActivationFunctionType.Exp,
                        scale=float(1.0 / np.sqrt(DK)),
                    )
                    for half in (0, 1):
                        kk = 2 * sl + half
                        d0 = d0s[half]
                        if 128 * kk >= q0:
                            # zero p where q < k inside the ragged diag block
                            nc.gpsimd.affine_select(
                                out=pt[:, half, d0 : d0 + 128],
                                in_=pt[:, half, d0 : d0 + 128],
                                compare_op=mybir.AluOpType.is_ge,
                                fill=0.0,
                                base=0,
                                pattern=[[1, 128]],
                                channel_multiplier=-1,
                            )
                    pts.append(pt)
                    if pieces:
                        n = (len(pieces) + (nsl - sl) - 1) // (nsl - sl)
                        for p in pieces[:n]:
                            p()
                        del pieces[:n]
                    drain()
                for qt in range(4):
                    pieces.append(pv_piece(s, h, qt, pts))

            emit_qkv_chunk0()
            for s in range(TCH):
                del fillers[:]
                if s == 0:
                    # V(0..3) rides the first slabs (PV lags 2 slabs, so
                    # vaug tiles 0/1 are only needed at the third slab)
                    for ti in range(4):
                        fillers.append(lambda ti=ti: emit_v(ti))
                if s + 1 < TCH:
                    for j in (0, 1, 2, 3):
                        fillers.append(lambda j=j, t=s + 1: emit_qk(j, t))
                    vmax = 2 if s + 1 == TCH - 1 else 4
                    for ti in range(4 * (s + 1), 4 * (s + 1) + vmax):
                        fillers.append(lambda ti=ti: emit_v(ti))
                if s == 3:
                    # late V tiles of the last seg, then earlier segs' W_O
                    for ti in range(14, 16):
                        fillers.insert(ti - 14, lambda ti=ti: emit_v(ti))
                    for ti in range(12):
                        fillers.append(lambda ti=ti: emit_wo(ti))
                nsl = 2 * (s + 1)
                pace["rate"] = len(fillers) / float(4 * nsl)
                pace["credit"] = 1.0 if s < TCH - 1 else pace["rate"]
                for h in range(HG):
                    emit_head(s, h)
                # safety: leftover fillers at seg end
                flushed = list(fillers)
                del fillers[:]
                for f in flushed:
                    f()
            for p in pieces:  # last head's chains (incl. the tail W_O)
                p()
            del pieces[:]


_CACHE = {}


def _build():
    if "nc" in _CACHE:
        return _CACHE["nc"]
    nc = bacc.Bacc("TRN2", debug=False, num_devices=N_CORES)
    xT = nc.dram_tensor("xT", [C, T], BF, kind="ExternalInput").ap()
    wqkvT = nc.dram_tensor("wqkvT", [C, 3 * WQ], BF, kind="ExternalInput").ap()
    woT = nc.dram_tensor("woT", [2 * 128, C], BF, kind="ExternalInput").ap()
    ident = nc.dram_tensor("ident", [128, 128], BF, kind="ExternalInput").ap()
    out = nc.dram_tensor("out", [T, C], F16, kind="ExternalOutput").ap()
    _emit(nc, xT, wqkvT, woT, ident, out)
    nc.compile()
    _CACHE["nc"] = nc
    return nc


_IDENT = np.eye(128, dtype=ml_dtypes.bfloat16)


def _shard_inputs(x, W_QKV, W_O):
    """Build the 8 per-core input maps. core = b*G + g."""
    in_maps = []
    W_Q, W_K, W_V = W_QKV[0:C], W_QKV[C : 2 * C], W_QKV[2 * C : 3 * C]
    xT_b = [
        np.ascontiguousarray(x[b].T).astype(ml_dtypes.bfloat16) for b in range(B)
    ]
    for b in range(B):
        for g in range(G):
            sl = slice(g * HG * DK, (g + 1) * HG * DK)
            w_g = np.concatenate([W_Q[sl], W_K[sl], W_V[sl]], axis=0)  # [768, C]
            wqkvT_g = np.ascontiguousarray(w_g.T).astype(ml_dtypes.bfloat16)
            woT_g = np.ascontiguousarray(W_O[:, sl].T).astype(ml_dtypes.bfloat16)
            in_maps.append(
                {"xT": xT_b[b], "wqkvT": wqkvT_g, "woT": woT_g, "ident": _IDENT}
            )
    return in_maps


def kernel(x, W_QKV, W_O):
    x = np.asarray(x, dtype=np.float32)
    W_QKV = np.asarray(W_QKV, dtype=np.float32)
    W_O = np.asarray(W_O, dtype=np.float32)
    nc = _build()
    in_maps = _shard_inputs(x, W_QKV, W_O)
    res = bass_utils.run_bass_kernel_spmd(
        nc, in_maps, core_ids=list(range(N_CORES))
    )
    out = np.zeros((B, T, C), dtype=np.float32)
    for b in range(B):
        for g in range(G):
            out[b] += np.asarray(res.results[b * G + g]["out"], dtype=np.float32)
    return out
